# revision 4
# baseline (speedup 1.0000x reference)
"""Trainium2 Bass kernel for nn_CustomLinearLayer:
    out = input @ (S * THETA).T + bias
with input [4096, 2048] f32, S/THETA [512, 2048] f32, bias [512] f32.

Strategy: data-parallel shard of the batch across 8 NeuronCores
(512 rows each); S/THETA/bias replicated. Per core:
  - DMA X shard with f32->f32r rounding cast (SWDGE), S/THETA natural f32
  - W = S * THETA elementwise on VectorE, written as f32r (rounds)
  - transpose X and W k-chunks on TensorE in f32r (single-pass),
    PSUM->SBUF copybacks split across VectorE/ScalarE
  - out.T[o, b] = sum_k wt[:, k, o-slice].T @ xt[:, k, :], f32r matmuls
    accumulated in fp32 PSUM, interleaved with the W-transpose stream
  - bias added during the PSUM->SBUF copyback (per-partition scalar add)
  - DMA out.T [512, 512] per core; host glue transposes/concats shards.
"""

import numpy as np

N_CORES = 8
BATCH, OUT_DIM, IN_DIM = 4096, 512, 2048
B_CORE = BATCH // N_CORES  # 512 batch rows per core
P = 128
KT = IN_DIM // P  # 16 k-tiles
BT = B_CORE // P  # 4 batch subtiles
OT = OUT_DIM // P  # 4 output subtiles

# "f32r_dma": X rounded to f32r during DMA; all transposes f32r
# "f32r": f32 transposes (LOW_HIGH), rounding on copyback
# "bf16": everything bf16 after DMA
MM_MODE = "f32r_dma"

_CACHE = {}


def _build(mode):
    from contextlib import ExitStack

    import concourse.bass as bass
    import concourse.tile as tile
    from concourse import bacc, mybir
    from concourse.masks import make_identity

    f32 = mybir.dt.float32
    f32r = mybir.dt.float32r
    bf16 = mybir.dt.bfloat16

    nc = bacc.Bacc("TRN2", target_bir_lowering=False, debug=False,
                   num_devices=N_CORES)

    x_d = nc.dram_tensor("x", [B_CORE, IN_DIM], f32, kind="ExternalInput").ap()
    s_d = nc.dram_tensor("s", [OUT_DIM, IN_DIM], f32, kind="ExternalInput").ap()
    th_d = nc.dram_tensor("th", [OUT_DIM, IN_DIM], f32, kind="ExternalInput").ap()
    # bias pre-arranged on host as [128, OT]: b[p, m] = bias[m*128 + p]
    b_d = nc.dram_tensor("b", [P, OT], f32, kind="ExternalInput").ap()
    # out.T layout: [OUT_DIM, B_CORE]
    o_d = nc.dram_tensor("o", [OUT_DIM, B_CORE], f32, kind="ExternalOutput").ap()

    if mode == "bf16":
        op_dt = bf16
    else:
        op_dt = f32r
    x_cast_in_dma = mode in ("f32r_dma", "bf16")

    with tile.TileContext(nc) as tc, ExitStack() as ctx:
        const = ctx.enter_context(tc.tile_pool(name="const", bufs=1))
        identity_f32 = const.tile([P, P], f32)
        make_identity(nc, identity_f32[:])
        identity = const.tile([P, P], op_dt)
        nc.vector.tensor_copy(identity[:], identity_f32[:])
        bias_col = const.tile([P, OT], f32)
        nc.sync.dma_start(bias_col[:], b_d[:])

        x_pool = ctx.enter_context(tc.tile_pool(name="x", bufs=2))
        s_pool = ctx.enter_context(tc.tile_pool(name="s", bufs=2))
        th_pool = ctx.enter_context(tc.tile_pool(name="th", bufs=2))
        w_pool = ctx.enter_context(tc.tile_pool(name="w", bufs=2))
        big = ctx.enter_context(tc.tile_pool(name="big", bufs=1))
        out_pool = ctx.enter_context(tc.tile_pool(name="out", bufs=2))
        tr_psum = ctx.enter_context(
            tc.tile_pool(name="trps", bufs=6, space="PSUM"))
        mm_psum = ctx.enter_context(
            tc.tile_pool(name="mmps", bufs=2, space="PSUM"))

        # transposed operands, resident: [k-part, k-tile, row]
        xt = big.tile([P, KT, B_CORE], op_dt)
        wt = big.tile([P, KT, OUT_DIM], op_dt)

        ncopy = 0

        def copyback(dst, src):
            # split PSUM->SBUF copybacks between VectorE and ScalarE
            nonlocal ncopy
            if ncopy % 2 == 0:
                nc.vector.tensor_copy(dst, src)
            else:
                nc.scalar.copy(dst, src)
            ncopy += 1

        # X path: load natural b-tiles (rounding to f32r in the DMA when
        # supported), transpose each k-chunk on PE
        x_tiles = []
        for bt in range(BT):
            x_t = x_pool.tile([P, IN_DIM], op_dt if x_cast_in_dma else f32)
            if x_cast_in_dma:
                nc.gpsimd.dma_start(x_t[:], x_d[bt * P:(bt + 1) * P, :])
            else:
                nc.sync.dma_start(x_t[:], x_d[bt * P:(bt + 1) * P, :])
            x_tiles.append(x_t)
        for bt in range(BT):
            x_t = x_tiles[bt]
            for k in range(KT):
                pt = tr_psum.tile([P, P], x_t.dtype)
                nc.tensor.transpose(pt[:], x_t[:, k * P:(k + 1) * P],
                                    identity[:])
                copyback(xt[:, k, bt * P:(bt + 1) * P], pt[:])

        # W path: load S/THETA o-tiles, multiply (rounds into op dtype),
        # transpose each k-chunk, and interleave this o-slice's matmuls
        # right behind its transposes (keeps PE warm, drains output early)
        for m in range(OT):
            s_t = s_pool.tile([P, IN_DIM], f32)
            nc.sync.dma_start(s_t[:], s_d[m * P:(m + 1) * P, :])
            th_t = th_pool.tile([P, IN_DIM], f32)
            nc.sync.dma_start(th_t[:], th_d[m * P:(m + 1) * P, :])
            w_t = w_pool.tile([P, IN_DIM], op_dt)
            nc.vector.tensor_mul(w_t[:], s_t[:], th_t[:])
            ps = mm_psum.tile([P, B_CORE], f32)
            for k in range(KT):
                pt = tr_psum.tile([P, P], op_dt)
                nc.tensor.transpose(pt[:], w_t[:, k * P:(k + 1) * P],
                                    identity[:])
                copyback(wt[:, k, m * P:(m + 1) * P], pt[:])
                nc.tensor.matmul(
                    ps[:],
                    wt[:, k, m * P:(m + 1) * P],
                    xt[:, k, :],
                    start=(k == 0),
                    stop=(k == KT - 1),
                )
            o_t = out_pool.tile([P, B_CORE], f32)
            # fused bias add: out.T[o, b] = psum[o, b] + bias[o]
            nc.vector.tensor_scalar_add(o_t[:], ps[:], bias_col[:, m:m + 1])
            nc.sync.dma_start(o_d[m * P:(m + 1) * P, :], o_t[:])

    nc.compile()
    return nc


def kernel(input, S, THETA, bias):
    from concourse.bass_utils import run_bass_kernel_spmd

    if MM_MODE not in _CACHE:
        _CACHE[MM_MODE] = _build(MM_MODE)
    nc = _CACHE[MM_MODE]

    input = np.ascontiguousarray(input, dtype=np.float32)
    S = np.ascontiguousarray(S, dtype=np.float32)
    THETA = np.ascontiguousarray(THETA, dtype=np.float32)
    bias = np.ascontiguousarray(bias, dtype=np.float32)
    b_host = np.ascontiguousarray(bias.reshape(OT, P).T)  # [128, OT]

    in_maps = [
        {
            "x": np.ascontiguousarray(input[c * B_CORE:(c + 1) * B_CORE]),
            "s": S,
            "th": THETA,
            "b": b_host,
        }
        for c in range(N_CORES)
    ]
    res = run_bass_kernel_spmd(nc, in_maps, core_ids=list(range(N_CORES)))
    out = np.empty((BATCH, OUT_DIM), dtype=np.float32)
    for c in range(N_CORES):
        out[c * B_CORE:(c + 1) * B_CORE, :] = res.results[c]["o"].T
    return out


# revision 6
# speedup vs baseline: 1.0509x; 1.0509x over previous
"""Trainium2 Bass kernel for nn_CustomLinearLayer:
    out = input @ (S * THETA).T + bias
with input [4096, 2048] f32, S/THETA [512, 2048] f32, bias [512] f32.

Strategy: data-parallel shard of the batch across 8 NeuronCores
(512 rows each); S/THETA/bias replicated. Per core:
  - DMA X shard with f32->f32r rounding cast (SWDGE), S/THETA natural f32
  - W = S * THETA elementwise on VectorE, written as f32r (rounds)
  - transpose X and W k-chunks on TensorE in f32r (single-pass),
    PSUM->SBUF copybacks split across VectorE/ScalarE
  - out.T[o, b] = sum_k wt[:, k, o-slice].T @ xt[:, k, :], f32r matmuls
    accumulated in fp32 PSUM, interleaved with the W-transpose stream
  - bias added during the PSUM->SBUF copyback (per-partition scalar add)
  - DMA out.T [512, 512] per core; host glue transposes/concats shards.
"""

import numpy as np

N_CORES = 8
BATCH, OUT_DIM, IN_DIM = 4096, 512, 2048
B_CORE = BATCH // N_CORES  # 512 batch rows per core
P = 128
KT = IN_DIM // P  # 16 k-tiles
BT = B_CORE // P  # 4 batch subtiles
OT = OUT_DIM // P  # 4 output subtiles

# "f32r_dma": X rounded to f32r during DMA; all transposes f32r
# "f32r": f32 transposes (LOW_HIGH), rounding on copyback
# "bf16": everything bf16 after DMA
MM_MODE = "f32r_dma"

_CACHE = {}


def _build(mode):
    from contextlib import ExitStack

    import concourse.bass as bass
    import concourse.tile as tile
    from concourse import bacc, mybir
    from concourse.masks import make_identity

    f32 = mybir.dt.float32
    f32r = mybir.dt.float32r
    bf16 = mybir.dt.bfloat16

    nc = bacc.Bacc("TRN2", target_bir_lowering=False, debug=False,
                   num_devices=N_CORES)

    x_d = nc.dram_tensor("x", [B_CORE, IN_DIM], f32, kind="ExternalInput").ap()
    s_d = nc.dram_tensor("s", [OUT_DIM, IN_DIM], f32, kind="ExternalInput").ap()
    th_d = nc.dram_tensor("th", [OUT_DIM, IN_DIM], f32, kind="ExternalInput").ap()
    # bias pre-arranged on host as [128, OT]: b[p, m] = bias[m*128 + p]
    b_d = nc.dram_tensor("b", [P, OT], f32, kind="ExternalInput").ap()
    # out.T layout: [OUT_DIM, B_CORE]
    o_d = nc.dram_tensor("o", [OUT_DIM, B_CORE], f32, kind="ExternalOutput").ap()

    if mode == "bf16":
        op_dt = bf16
    else:
        op_dt = f32r
    x_cast_in_dma = mode in ("f32r_dma", "bf16")

    with tile.TileContext(nc) as tc, ExitStack() as ctx:
        const = ctx.enter_context(tc.tile_pool(name="const", bufs=1))
        identity_f32 = const.tile([P, P], f32)
        make_identity(nc, identity_f32[:])
        identity = const.tile([P, P], op_dt)
        nc.vector.tensor_copy(identity[:], identity_f32[:])
        bias_col = const.tile([P, OT], f32)
        nc.sync.dma_start(bias_col[:], b_d[:])

        x_pool = ctx.enter_context(tc.tile_pool(name="x", bufs=4))
        s_pool = ctx.enter_context(tc.tile_pool(name="s", bufs=2))
        th_pool = ctx.enter_context(tc.tile_pool(name="th", bufs=2))
        w_pool = ctx.enter_context(tc.tile_pool(name="w", bufs=2))
        big = ctx.enter_context(tc.tile_pool(name="big", bufs=1))
        out_pool = ctx.enter_context(tc.tile_pool(name="out", bufs=2))
        tr_psum = ctx.enter_context(
            tc.tile_pool(name="trps", bufs=6, space="PSUM"))
        mm_psum = ctx.enter_context(
            tc.tile_pool(name="mmps", bufs=2, space="PSUM"))

        # transposed operands, resident: [k-part, k-tile, row]
        xt = big.tile([P, KT, B_CORE], op_dt)
        wt = big.tile([P, KT, OUT_DIM], op_dt)

        ncopy = 0

        def copyback(dst, src):
            # split PSUM->SBUF copybacks between VectorE and ScalarE
            nonlocal ncopy
            if ncopy % 2 == 0:
                nc.vector.tensor_copy(dst, src)
            else:
                nc.scalar.copy(dst, src)
            ncopy += 1

        # X path: load natural b-tiles (rounding to f32r in the SWDGE DMA),
        # then transpose each k-chunk on PE
        x_tiles = []
        for bt in range(BT):
            x_t = x_pool.tile([P, IN_DIM], op_dt if x_cast_in_dma else f32)
            if x_cast_in_dma:
                nc.gpsimd.dma_start(x_t[:], x_d[bt * P:(bt + 1) * P, :])
            else:
                nc.sync.dma_start(x_t[:], x_d[bt * P:(bt + 1) * P, :])
            x_tiles.append(x_t)
        for bt in range(BT):
            x_t = x_tiles[bt]
            for k in range(KT):
                pt = tr_psum.tile([P, P], x_t.dtype)
                nc.tensor.transpose(pt[:], x_t[:, k * P:(k + 1) * P],
                                    identity[:])
                copyback(xt[:, k, bt * P:(bt + 1) * P], pt[:])

        # W path: load S/THETA o-tiles (split across both HWDGE rings),
        # multiply (rounds into op dtype), transpose each k-chunk; the
        # o-slice's 16 matmuls follow its 16 transposes, so the PE keeps a
        # dense [tr x16, mm x16] rhythm and output drains early
        for m in range(OT):
            s_t = s_pool.tile([P, IN_DIM], f32)
            nc.sync.dma_start(s_t[:], s_d[m * P:(m + 1) * P, :])
            th_t = th_pool.tile([P, IN_DIM], f32)
            nc.scalar.dma_start(th_t[:], th_d[m * P:(m + 1) * P, :])
            w_t = w_pool.tile([P, IN_DIM], op_dt)
            nc.vector.tensor_mul(w_t[:], s_t[:], th_t[:])
            for k in range(KT):
                pt = tr_psum.tile([P, P], op_dt)
                nc.tensor.transpose(pt[:], w_t[:, k * P:(k + 1) * P],
                                    identity[:])
                copyback(wt[:, k, m * P:(m + 1) * P], pt[:])
            ps = mm_psum.tile([P, B_CORE], f32)
            for k in range(KT):
                nc.tensor.matmul(
                    ps[:],
                    wt[:, k, m * P:(m + 1) * P],
                    xt[:, k, :],
                    start=(k == 0),
                    stop=(k == KT - 1),
                )
            o_t = out_pool.tile([P, B_CORE], f32)
            # fused bias add: out.T[o, b] = psum[o, b] + bias[o]
            nc.vector.tensor_scalar_add(o_t[:], ps[:], bias_col[:, m:m + 1])
            nc.scalar.dma_start(o_d[m * P:(m + 1) * P, :], o_t[:])

    nc.compile()
    return nc


def kernel(input, S, THETA, bias):
    from concourse.bass_utils import run_bass_kernel_spmd

    if MM_MODE not in _CACHE:
        _CACHE[MM_MODE] = _build(MM_MODE)
    nc = _CACHE[MM_MODE]

    input = np.ascontiguousarray(input, dtype=np.float32)
    S = np.ascontiguousarray(S, dtype=np.float32)
    THETA = np.ascontiguousarray(THETA, dtype=np.float32)
    bias = np.ascontiguousarray(bias, dtype=np.float32)
    b_host = np.ascontiguousarray(bias.reshape(OT, P).T)  # [128, OT]

    in_maps = [
        {
            "x": np.ascontiguousarray(input[c * B_CORE:(c + 1) * B_CORE]),
            "s": S,
            "th": THETA,
            "b": b_host,
        }
        for c in range(N_CORES)
    ]
    res = run_bass_kernel_spmd(nc, in_maps, core_ids=list(range(N_CORES)))
    out = np.empty((BATCH, OUT_DIM), dtype=np.float32)
    for c in range(N_CORES):
        out[c * B_CORE:(c + 1) * B_CORE, :] = res.results[c]["o"].T
    return out


# revision 7
# speedup vs baseline: 1.1552x; 1.0993x over previous
"""Trainium2 Bass kernel for nn_CustomLinearLayer:
    out = input @ (S * THETA).T + bias
with input [4096, 2048] f32, S/THETA [512, 2048] f32, bias [512] f32.

Strategy: data-parallel shard of the batch across 8 NeuronCores
(512 rows each); S/THETA/bias replicated. Per core:
  - DMA X shard with f32->f32r rounding cast (SWDGE), S/THETA natural f32
  - W = S * THETA elementwise on VectorE, written as f32r (rounds)
  - transpose X and W k-chunks on TensorE in f32r (single-pass),
    PSUM->SBUF copybacks split across VectorE/ScalarE
  - out.T[o, b] = sum_k wt[:, k, o-slice].T @ xt[:, k, :], f32r matmuls
    accumulated in fp32 PSUM, interleaved with the W-transpose stream
  - bias added during the PSUM->SBUF copyback (per-partition scalar add)
  - DMA out.T [512, 512] per core; host glue transposes/concats shards.
"""

import numpy as np

N_CORES = 8
BATCH, OUT_DIM, IN_DIM = 4096, 512, 2048
B_CORE = BATCH // N_CORES  # 512 batch rows per core
P = 128
KT = IN_DIM // P  # 16 k-tiles
BT = B_CORE // P  # 4 batch subtiles
OT = OUT_DIM // P  # 4 output subtiles

# "f32r_dma": X rounded to f32r during DMA; all transposes f32r
# "f32r": f32 transposes (LOW_HIGH), rounding on copyback
# "bf16": everything bf16 after DMA
MM_MODE = "f32r_dma"

_CACHE = {}


def _build(mode):
    from contextlib import ExitStack

    import concourse.bass as bass
    import concourse.tile as tile
    from concourse import bacc, mybir
    from concourse.masks import make_identity

    f32 = mybir.dt.float32
    f32r = mybir.dt.float32r
    bf16 = mybir.dt.bfloat16

    nc = bacc.Bacc("TRN2", target_bir_lowering=False, debug=False,
                   num_devices=N_CORES)

    x_d = nc.dram_tensor("x", [B_CORE, IN_DIM], f32, kind="ExternalInput").ap()
    s_d = nc.dram_tensor("s", [OUT_DIM, IN_DIM], f32, kind="ExternalInput").ap()
    th_d = nc.dram_tensor("th", [OUT_DIM, IN_DIM], f32, kind="ExternalInput").ap()
    # bias pre-arranged on host as [128, OT]: b[p, m] = bias[m*128 + p]
    b_d = nc.dram_tensor("b", [P, OT], f32, kind="ExternalInput").ap()
    # out.T layout: [OUT_DIM, B_CORE]
    o_d = nc.dram_tensor("o", [OUT_DIM, B_CORE], f32, kind="ExternalOutput").ap()

    if mode == "bf16":
        op_dt = bf16
    else:
        op_dt = f32r
    x_cast_in_dma = mode in ("f32r_dma", "bf16")

    with tile.TileContext(nc) as tc, ExitStack() as ctx:
        const = ctx.enter_context(tc.tile_pool(name="const", bufs=1))
        identity_f32 = const.tile([P, P], f32)
        make_identity(nc, identity_f32[:])
        identity = const.tile([P, P], op_dt)
        nc.vector.tensor_copy(identity[:], identity_f32[:])
        bias_col = const.tile([P, OT], f32)
        nc.sync.dma_start(bias_col[:], b_d[:])

        x_pool = ctx.enter_context(tc.tile_pool(name="x", bufs=4))
        s_pool = ctx.enter_context(tc.tile_pool(name="s", bufs=2))
        th_pool = ctx.enter_context(tc.tile_pool(name="th", bufs=2))
        w_pool = ctx.enter_context(tc.tile_pool(name="w", bufs=2))
        big = ctx.enter_context(tc.tile_pool(name="big", bufs=1))
        out_pool = ctx.enter_context(tc.tile_pool(name="out", bufs=2))
        tr_psum = ctx.enter_context(
            tc.tile_pool(name="trps", bufs=6, space="PSUM"))
        mm_psum = ctx.enter_context(
            tc.tile_pool(name="mmps", bufs=2, space="PSUM"))

        # transposed operands, resident: [k-part, k-tile, row]
        xt = big.tile([P, KT, B_CORE], op_dt)
        wt = big.tile([P, KT, OUT_DIM], op_dt)

        ncopy = 0

        def copyback(dst, src):
            # split PSUM->SBUF copybacks between VectorE and ScalarE
            nonlocal ncopy
            if ncopy % 2 == 0:
                nc.vector.tensor_copy(dst, src)
            else:
                nc.scalar.copy(dst, src)
            ncopy += 1

        def transpose4(dst4, src_t, k0):
            # transpose 4 consecutive k-chunks into one PSUM bank, then one
            # wide copyback (amortizes per-op overheads on DVE/ScalarE)
            pt = tr_psum.tile([P, 4 * P], op_dt)
            for j in range(4):
                nc.tensor.matmul(
                    pt[:, j * P:(j + 1) * P],
                    src_t[:, (k0 + j) * P:(k0 + j + 1) * P],
                    identity[:],
                    is_transpose=True,
                    start=(j == 0),
                    stop=(j == 3),
                )
            copyback(dst4, pt[:])

        # X path: load natural b-tiles f32 on the sync HWDGE ring, round to
        # f32r on DVE/ScalarE, then transpose k-chunks on PE (single-pass
        # f32r transposes, 4 per PSUM bank)
        x_tiles = []
        for bt in range(BT):
            x_f = x_pool.tile([P, IN_DIM], f32, tag="x_f32")
            nc.sync.dma_start(x_f[:], x_d[bt * P:(bt + 1) * P, :])
            x_t = x_pool.tile([P, IN_DIM], op_dt, tag="x_op")
            if bt % 2 == 0:
                nc.vector.tensor_copy(x_t[:], x_f[:])
            else:
                nc.scalar.copy(x_t[:], x_f[:])
            x_tiles.append(x_t)
        for bt in range(BT):
            x_t = x_tiles[bt]
            for k0 in range(0, KT, 4):
                # dst: xt[:, k0:k0+4, bt-slice] is [128, 4, 128]
                transpose4(xt[:, k0:k0 + 4, bt * P:(bt + 1) * P], x_t, k0)

        # W path: load S/THETA o-tiles (split across both HWDGE rings),
        # multiply (rounds into op dtype), transpose k-chunks; the o-slice's
        # 16 matmuls follow its 16 transposes, so the PE keeps a dense
        # [tr x16, mm x16] rhythm and output drains early
        for m in range(OT):
            s_t = s_pool.tile([P, IN_DIM], f32)
            nc.sync.dma_start(s_t[:], s_d[m * P:(m + 1) * P, :])
            th_t = th_pool.tile([P, IN_DIM], f32)
            nc.scalar.dma_start(th_t[:], th_d[m * P:(m + 1) * P, :])
            w_t = w_pool.tile([P, IN_DIM], op_dt)
            nc.vector.tensor_mul(w_t[:], s_t[:], th_t[:])
            for k0 in range(0, KT, 4):
                transpose4(wt[:, k0:k0 + 4, m * P:(m + 1) * P], w_t, k0)
            ps = mm_psum.tile([P, B_CORE], f32)
            for k in range(KT):
                nc.tensor.matmul(
                    ps[:],
                    wt[:, k, m * P:(m + 1) * P],
                    xt[:, k, :],
                    start=(k == 0),
                    stop=(k == KT - 1),
                )
            o_t = out_pool.tile([P, B_CORE], f32)
            # fused bias add: out.T[o, b] = psum[o, b] + bias[o]
            nc.vector.tensor_scalar_add(o_t[:], ps[:], bias_col[:, m:m + 1])
            nc.scalar.dma_start(o_d[m * P:(m + 1) * P, :], o_t[:])

    nc.compile()
    return nc


def kernel(input, S, THETA, bias):
    from concourse.bass_utils import run_bass_kernel_spmd

    if MM_MODE not in _CACHE:
        _CACHE[MM_MODE] = _build(MM_MODE)
    nc = _CACHE[MM_MODE]

    input = np.ascontiguousarray(input, dtype=np.float32)
    S = np.ascontiguousarray(S, dtype=np.float32)
    THETA = np.ascontiguousarray(THETA, dtype=np.float32)
    bias = np.ascontiguousarray(bias, dtype=np.float32)
    b_host = np.ascontiguousarray(bias.reshape(OT, P).T)  # [128, OT]

    in_maps = [
        {
            "x": np.ascontiguousarray(input[c * B_CORE:(c + 1) * B_CORE]),
            "s": S,
            "th": THETA,
            "b": b_host,
        }
        for c in range(N_CORES)
    ]
    res = run_bass_kernel_spmd(nc, in_maps, core_ids=list(range(N_CORES)))
    out = np.empty((BATCH, OUT_DIM), dtype=np.float32)
    for c in range(N_CORES):
        out[c * B_CORE:(c + 1) * B_CORE, :] = res.results[c]["o"].T
    return out
